# revision 3
# baseline (speedup 1.0000x reference)
"""Trainium2 Bass kernel for nn_DecoderRNN (2-layer GRU decoder, teacher forcing).

Strategy (8 NeuronCores, no collectives):
  - Data-parallel over batch: each core runs the full 127-step GRU recurrence
    for its 4 of 32 sequences (B_loc=4).  Everything is kept in a transposed
    layout ([hidden-dim on partitions, batch on free]) so the gate math runs
    on all 128 partitions and the hidden state never needs transposing.
  - Input-side matmuls (embedding @ W_ih0, h0' @ W_ih1) are batched over
    32-step windows (M=128) for full PE utilisation; only the two h @ W_hh
    matmuls run per-step (weight-stationary, N=4, FWL bf16).
  - Output projection is row-sharded (each core projects its own 508 token
    rows over the full 32000 vocab) with a streaming online log-softmax, so
    log_softmax is fully local.  out_W^T is streamed from HBM in two passes
    (2 token-tiles per pass) so it is only read twice per core.

kernel(**inputs) takes the FULL unsharded inputs and returns the full
[127, 32, 32000] float32 log-prob tensor.
"""

import sys
import time

import numpy as np

try:
    import concourse  # noqa: F401
except ImportError:  # pragma: no cover
    sys.path.insert(0, "/opt/trn_rl_repo")

import ml_dtypes

import concourse.bass as bass
import concourse.tile as tile
from concourse import bacc, mybir
from concourse.bass import ds
from concourse.masks import make_identity

BF16 = mybir.dt.bfloat16
F32 = mybir.dt.float32
I32 = mybir.dt.int32
AF = mybir.ActivationFunctionType
PE = mybir.EngineType.PE

# Problem constants (hardcoded per contract).
V, E, H = 32000, 1024, 1024
B, T = 32, 128
NSTEP = T - 1          # 127 real recurrence steps
NCORES = 8
BLOC = B // NCORES     # 4 sequences per core
WSTEPS = 32            # recurrence steps per window
NW = 4                 # windows (128 steps incl. 1 pad step)
TOKW = WSTEPS * BLOC   # 128 local tokens per window
LTOK = NW * TOKW       # 512 local tokens (incl. pad)
G3 = 3 * H             # 3072 gate rows (r, z, n)
KC = H // 128          # 8 contraction chunks
MG = G3 // 128         # 24 gate-row groups of 128
HCW = WSTEPS * BLOC + BLOC   # 132 cols per h-chunk (4 carry + 32*4 state)
VCH = 500              # vocab chunk
NVC = V // VCH         # 64 vocab chunks
TT_PER_PASS = 2        # token-tiles sharing one out_W stream pass

_np_bf16 = ml_dtypes.bfloat16

_CACHE = {}


def _bf(x):
    return np.ascontiguousarray(np.asarray(x, dtype=np.float32).astype(_np_bf16))


def _wtiles(W):
    """[3H, H] weight -> SBUF lhsT layout [128, MG*KC*128] (bf16).

    sb[p, (m*KC+k)*128 + q] = W[128m + q, 128k + p]  (lhsT tile (m,k) = W-block^T)
    """
    W4 = np.asarray(W, dtype=np.float32).reshape(MG, 128, KC, 128)  # m q k p
    return _bf(W4.transpose(3, 0, 2, 1).reshape(128, MG * KC * 128))


def _build_program():
    nc = bacc.Bacc("TRN2", target_bir_lowering=False, debug=False,
                   num_devices=NCORES)

    # ---- DRAM I/O ----
    embW = nc.dram_tensor("embW", [V, E], F32, kind="ExternalInput")
    idx = nc.dram_tensor("idx", [LTOK, 1], I32, kind="ExternalInput")
    wih0 = nc.dram_tensor("wih0", [128, MG * KC * 128], BF16, kind="ExternalInput")
    whh0 = nc.dram_tensor("whh0", [128, MG * KC * 128], BF16, kind="ExternalInput")
    wih1 = nc.dram_tensor("wih1", [128, MG * KC * 128], BF16, kind="ExternalInput")
    whh1 = nc.dram_tensor("whh1", [128, MG * KC * 128], BF16, kind="ExternalInput")
    # biases: [1, G3] combined input-side bias per layer, [1, H] hidden n-bias
    bg0 = nc.dram_tensor("bg0", [1, G3], BF16, kind="ExternalInput")
    bn0 = nc.dram_tensor("bn0", [1, H], BF16, kind="ExternalInput")
    bg1 = nc.dram_tensor("bg1", [1, G3], BF16, kind="ExternalInput")
    bn1 = nc.dram_tensor("bn1", [1, H], BF16, kind="ExternalInput")
    hinit = nc.dram_tensor("hinit", [2, H, BLOC], BF16, kind="ExternalInput")
    woutT = nc.dram_tensor("woutT", [H, V], BF16, kind="ExternalInput")
    outb = nc.dram_tensor("outb", [1, V], BF16, kind="ExternalInput")
    out = nc.dram_tensor("out", [LTOK, V], F32, kind="ExternalOutput")

    with tile.TileContext(nc) as tc:
        with (
            tc.tile_pool(name="const", bufs=1) as cpool,
            tc.tile_pool(name="psum_t", bufs=2, space="PSUM") as tpsum,
        ):
            ident = cpool.tile([128, 128], F32)
            make_identity(nc, ident[:])
            ones = cpool.tile([1, 128], BF16)
            nc.gpsimd.memset(ones[:], 1.0)
            bg0_s = cpool.tile([1, G3], BF16)
            nc.sync.dma_start(bg0_s[:], bg0[:, :])
            bn0_s = cpool.tile([1, H], BF16)
            nc.sync.dma_start(bn0_s[:], bn0[:, :])
            bg1_s = cpool.tile([1, G3], BF16)
            nc.sync.dma_start(bg1_s[:], bg1[:, :])
            bn1_s = cpool.tile([1, H], BF16)
            nc.sync.dma_start(bn1_s[:], bn1[:, :])
            # h1 history for the projection: chunk c cols [512c : 512c+512]
            h1all = cpool.tile([128, KC * LTOK], BF16)

            # ================= GRU phase =================
            with (
                tc.tile_pool(name="gruw", bufs=1) as gw,
                tc.tile_pool(name="gwin", bufs=1) as gwin,
                tc.tile_pool(name="wstr", bufs=8) as wstr,
                tc.tile_pool(name="xemb", bufs=1) as xpool,
                tc.tile_pool(name="scr", bufs=2) as scr,
                tc.tile_pool(name="psum_b", bufs=2, space="PSUM") as bpsum,
                tc.tile_pool(name="psum_g", bufs=1, space="PSUM") as gpsum,
            ):
                whh0_s = gw.tile([128, MG * KC * 128], BF16, tag="whh0")
                nc.sync.dma_start(whh0_s[:], whh0[:, :])
                wih1_s = gw.tile([128, MG * KC * 128], BF16, tag="wih1")
                nc.sync.dma_start(wih1_s[:], wih1[:, :])
                whh1_s = gw.tile([128, MG * KC * 128], BF16, tag="whh1")
                nc.sync.dma_start(whh1_s[:], whh1[:, :])

                h0T = gwin.tile([128, KC * HCW], BF16, tag="h0T")
                h1T = gwin.tile([128, KC * HCW], BF16, tag="h1T")

                def sweep(wsb, giT, hT, bn_s):
                    """32 recurrence steps of one GRU layer (transposed layout)."""
                    ghp = gpsum.tile([128, MG * BLOC], F32, tag="ghp")
                    ghv = ghp[:].rearrange("p (m q) -> p m q", q=BLOC)
                    giv = giT[:].rearrange("p (m q) -> p m q", q=TOKW)
                    hv = hT[:].rearrange("p (c q) -> p c q", q=HCW)

                    def body(s):
                        for m in range(MG):
                            for k in range(KC):
                                nc.tensor.matmul(
                                    ghp[:, BLOC * m:BLOC * (m + 1)],
                                    lhsT=wsb[:, (m * KC + k) * 128:(m * KC + k + 1) * 128],
                                    rhs=hT[:, ds(k * HCW + s * BLOC, BLOC)],
                                    start=(k == 0),
                                    stop=(k == KC - 1 and m < 2 * KC))
                            if m >= 2 * KC:  # n-gate rows: add b_hh_n
                                nc.tensor.matmul(
                                    ghp[:, BLOC * m:BLOC * (m + 1)],
                                    lhsT=bn_s[0:1, 128 * (m - 2 * KC):128 * (m - 2 * KC + 1)],
                                    rhs=ones[0:1, 0:BLOC], start=False, stop=True)
                        # gates  (all [128, m-groups, 4] views)
                        rz = scr.tile([128, 16 * BLOC], F32, tag="rz")
                        rzv = rz[:].rearrange("p (m q) -> p m q", q=BLOC)
                        nc.vector.tensor_add(rzv[:, :, :], giv[:, 0:16, ds(s * BLOC, BLOC)],
                                             ghv[:, 0:16, :])
                        rza = scr.tile([128, 16 * BLOC], F32, tag="rza")
                        nc.scalar.activation(rza[:], rz[:], AF.Sigmoid)
                        rzav = rza[:].rearrange("p (m q) -> p m q", q=BLOC)
                        rn = scr.tile([128, 8 * BLOC], F32, tag="rn")
                        rnv = rn[:].rearrange("p (m q) -> p m q", q=BLOC)
                        nc.vector.tensor_mul(rnv[:, :, :], rzav[:, 0:8, :], ghv[:, 16:24, :])
                        t3 = scr.tile([128, 8 * BLOC], F32, tag="t3")
                        t3v = t3[:].rearrange("p (m q) -> p m q", q=BLOC)
                        nc.vector.tensor_add(t3v[:, :, :], rnv[:, :, :],
                                             giv[:, 16:24, ds(s * BLOC, BLOC)])
                        na = scr.tile([128, 8 * BLOC], F32, tag="na")
                        nc.scalar.activation(na[:], t3[:], AF.Tanh)
                        nav = na[:].rearrange("p (m q) -> p m q", q=BLOC)
                        d = scr.tile([128, 8 * BLOC], F32, tag="d")
                        dv = d[:].rearrange("p (m q) -> p m q", q=BLOC)
                        nc.vector.tensor_sub(dv[:, :, :], hv[:, :, ds(s * BLOC, BLOC)],
                                             nav[:, :, :])
                        zd = scr.tile([128, 8 * BLOC], F32, tag="zd")
                        zdv = zd[:].rearrange("p (m q) -> p m q", q=BLOC)
                        nc.vector.tensor_mul(zdv[:, :, :], rzav[:, 8:16, :], dv[:, :, :])
                        nc.vector.tensor_add(hv[:, :, ds(s * BLOC + BLOC, BLOC)],
                                             nav[:, :, :], zdv[:, :, :])

                    tc.For_i_unrolled_general(
                        start=0, end=WSTEPS, step=1,
                        unrollable_body=lambda iv, u: [body(iv + i) for i in range(u)],
                        max_unroll=4, hint_engines=(PE,))

                for w in range(NW):
                    # ---- gather + transpose this window's embeddings ----
                    idxt = xpool.tile([128, 1], I32, tag="idxt")
                    nc.sync.dma_start(idxt[:], idx[TOKW * w:TOKW * (w + 1), :])
                    xg = xpool.tile([128, E], F32, tag="xg")
                    nc.gpsimd.indirect_dma_start(
                        out=xg[:], out_offset=None, in_=embW[:, :],
                        in_offset=bass.IndirectOffsetOnAxis(ap=idxt[:, 0:1], axis=0))
                    xT = xpool.tile([128, KC * 128], BF16, tag="xT")
                    for c in range(KC):
                        pt = tpsum.tile([128, 128], F32, tag="pt")
                        nc.tensor.transpose(out=pt[:], in_=xg[:, 128 * c:128 * (c + 1)],
                                            identity=ident[:])
                        nc.scalar.copy(out=xT[:, 128 * c:128 * (c + 1)], in_=pt[:])
                    # ---- carry-in hidden state ----
                    for l, hT_l in ((0, h0T), (1, h1T)):
                        hv_l = hT_l[:].rearrange("p (c q) -> p c q", q=HCW)
                        if w == 0:
                            nc.sync.dma_start(
                                hv_l[:, :, 0:BLOC],
                                hinit[l, :, :].rearrange("(c p) q -> p c q", p=128))
                        else:
                            nc.vector.tensor_copy(
                                hv_l[:, :, 0:BLOC],
                                hv_l[:, :, WSTEPS * BLOC:WSTEPS * BLOC + BLOC])
                    # ---- layer 0 ----
                    giT = gwin.tile([128, MG * TOKW], BF16, tag="giT")
                    for m in range(MG):
                        pb = bpsum.tile([128, 128], F32, tag="pb")
                        for k in range(KC):
                            wt = wstr.tile([128, 128], BF16, tag="wt")
                            nc.sync.dma_start(
                                wt[:], wih0[:, (m * KC + k) * 128:(m * KC + k + 1) * 128])
                            nc.tensor.matmul(pb[:], lhsT=wt[:],
                                             rhs=xT[:, 128 * k:128 * (k + 1)],
                                             start=(k == 0), stop=False)
                        nc.tensor.matmul(pb[:], lhsT=bg0_s[0:1, 128 * m:128 * (m + 1)],
                                         rhs=ones[0:1, 0:128], start=False, stop=True)
                        nc.scalar.copy(out=giT[:, 128 * m:128 * (m + 1)], in_=pb[:])
                    sweep(whh0_s, giT, h0T, bn0_s)
                    # ---- layer 1 ----
                    giT = gwin.tile([128, MG * TOKW], BF16, tag="giT")
                    for m in range(MG):
                        pb = bpsum.tile([128, 128], F32, tag="pb")
                        for k in range(KC):
                            nc.tensor.matmul(
                                pb[:],
                                lhsT=wih1_s[:, (m * KC + k) * 128:(m * KC + k + 1) * 128],
                                rhs=h0T[:, k * HCW + BLOC:k * HCW + BLOC + TOKW],
                                start=(k == 0), stop=False)
                        nc.tensor.matmul(pb[:], lhsT=bg1_s[0:1, 128 * m:128 * (m + 1)],
                                         rhs=ones[0:1, 0:128], start=False, stop=True)
                        nc.scalar.copy(out=giT[:, 128 * m:128 * (m + 1)], in_=pb[:])
                    sweep(whh1_s, giT, h1T, bn1_s)
                    # ---- stash h1 history for the projection ----
                    for c in range(KC):
                        nc.scalar.copy(
                            out=h1all[:, c * LTOK + TOKW * w:c * LTOK + TOKW * (w + 1)],
                            in_=h1T[:, c * HCW + BLOC:c * HCW + BLOC + TOKW])

            # ================= projection phase =================
            with (
                tc.tile_pool(name="wout", bufs=6) as wpool,
                tc.tile_pool(name="bstr", bufs=2) as bpool,
                tc.tile_pool(name="xs", bufs=1) as xspool,
                tc.tile_pool(name="stat", bufs=2) as spool,
                tc.tile_pool(name="ostg", bufs=4) as opool,
                tc.tile_pool(name="psum_p", bufs=2, space="PSUM") as ppsum,
            ):
                for half in range(LTOK // TOKW // TT_PER_PASS):
                    tts = [half * TT_PER_PASS + i for i in range(TT_PER_PASS)]
                    xs = {t: xspool.tile([128, V], BF16, tag=f"xs{t % TT_PER_PASS}", name=f"xs{t}")
                          for t in tts}
                    m_arr = {t: spool.tile([128, NVC], F32, tag=f"ma{t % TT_PER_PASS}", name=f"ma{t}")
                             for t in tts}
                    s_arr = {t: spool.tile([128, NVC], F32, tag=f"sa{t % TT_PER_PASS}", name=f"sa{t}")
                             for t in tts}
                    for v in range(NVC):
                        wts = []
                        for k in range(KC):
                            wt = wpool.tile([128, VCH], BF16, tag="wt", name="wtp")
                            nc.sync.dma_start(
                                wt[:], woutT[128 * k:128 * (k + 1), VCH * v:VCH * (v + 1)])
                            wts.append(wt)
                        bt = bpool.tile([1, VCH], BF16, tag="bt")
                        nc.sync.dma_start(bt[:], outb[0:1, VCH * v:VCH * (v + 1)])
                        for t in tts:
                            pp = ppsum.tile([128, VCH], F32, tag="pp")
                            for k in range(KC):
                                nc.tensor.matmul(
                                    pp[:],
                                    lhsT=h1all[:, k * LTOK + TOKW * t:k * LTOK + TOKW * (t + 1)],
                                    rhs=wts[k][:], start=(k == 0), stop=False)
                            nc.tensor.matmul(pp[:], lhsT=ones[0:1, 0:128], rhs=bt[0:1, :],
                                             start=False, stop=True)
                            nc.vector.reduce_max(m_arr[t][:, v:v + 1], pp[:],
                                                 axis=mybir.AxisListType.X)
                            negm = spool.tile([128, 1], F32, tag="negm")
                            nc.vector.tensor_scalar_mul(negm[:], m_arr[t][:, v:v + 1], -1.0)
                            nc.scalar.activation(xs[t][:, VCH * v:VCH * (v + 1)], pp[:],
                                                 AF.Identity, bias=negm[:])
                            es = spool.tile([128, VCH], F32, tag="es")
                            nc.scalar.activation(es[:], pp[:], AF.Exp, bias=negm[:],
                                                 accum_out=s_arr[t][:, v:v + 1])
                    for t in tts:
                        M = spool.tile([128, 1], F32, tag="M")
                        nc.vector.reduce_max(M[:], m_arr[t][:], axis=mybir.AxisListType.X)
                        negM = spool.tile([128, 1], F32, tag="negM")
                        nc.vector.tensor_scalar_mul(negM[:], M[:], -1.0)
                        ee = spool.tile([128, NVC], F32, tag="ee")
                        nc.scalar.activation(ee[:], m_arr[t][:], AF.Exp, bias=negM[:])
                        se = spool.tile([128, NVC], F32, tag="se")
                        nc.vector.tensor_mul(se[:], s_arr[t][:], ee[:])
                        sf = spool.tile([128, 1], F32, tag="sf")
                        nc.vector.reduce_sum(sf[:], se[:], axis=mybir.AxisListType.X)
                        lse = spool.tile([128, 1], F32, tag="lse")
                        nc.scalar.activation(lse[:], sf[:], AF.Ln)
                        nc.vector.tensor_add(lse[:], lse[:], M[:])
                        delta = spool.tile([128, NVC], F32, tag="delta")
                        nc.vector.tensor_scalar_sub(delta[:], m_arr[t][:], lse[:])
                        for v in range(NVC):
                            og = opool.tile([128, VCH], F32, tag="og")
                            nc.scalar.activation(og[:], xs[t][:, VCH * v:VCH * (v + 1)],
                                                 AF.Identity, bias=delta[:, v:v + 1])
                            nc.sync.dma_start(
                                out[TOKW * t:TOKW * (t + 1), VCH * v:VCH * (v + 1)], og[:])

    nc.compile()
    return nc


def _prep_inputs(inputs):
    target = np.asarray(inputs["target"]).astype(np.int32)          # [B, T]
    enc = np.asarray(inputs["encoder_hidden"], dtype=np.float32)    # [2, B, H]
    emb_W = np.ascontiguousarray(np.asarray(inputs["emb_W"], dtype=np.float32))
    W_ih = np.asarray(inputs["W_ih"], dtype=np.float32)             # [2, 3H, E]
    W_hh = np.asarray(inputs["W_hh"], dtype=np.float32)             # [2, 3H, H]
    b_ih = np.asarray(inputs["b_ih"], dtype=np.float32)             # [2, 3H]
    b_hh = np.asarray(inputs["b_hh"], dtype=np.float32)             # [2, 3H]
    out_W = np.asarray(inputs["out_W"], dtype=np.float32)           # [V, H]
    out_b = np.asarray(inputs["out_b"], dtype=np.float32)           # [V]

    shared = {
        "embW": emb_W,
        "wih0": _wtiles(W_ih[0]), "whh0": _wtiles(W_hh[0]),
        "wih1": _wtiles(W_ih[1]), "whh1": _wtiles(W_hh[1]),
        "woutT": _bf(np.ascontiguousarray(out_W.T)),
        "outb": _bf(out_b.reshape(1, V)),
    }
    for l, (nm_g, nm_n) in enumerate((("bg0", "bn0"), ("bg1", "bn1"))):
        bg = b_ih[l].copy()
        bg[:2 * H] += b_hh[l][:2 * H]
        shared[nm_g] = _bf(bg.reshape(1, G3))
        shared[nm_n] = _bf(b_hh[l][2 * H:].reshape(1, H))

    in_maps = []
    tpad = np.minimum(np.arange(NW * WSTEPS), NSTEP - 1)  # step 127 -> pad
    valid = np.arange(NW * WSTEPS) < NSTEP
    for k in range(NCORES):
        rows = target[BLOC * k:BLOC * (k + 1)]            # [4, T]
        idx = np.where(valid[:, None], rows.T[tpad], 0)   # [128, 4]
        m = dict(shared)
        m["idx"] = np.ascontiguousarray(idx.reshape(LTOK, 1).astype(np.int32))
        m["hinit"] = _bf(enc[:, BLOC * k:BLOC * (k + 1), :].transpose(0, 2, 1))
        in_maps.append(m)
    return in_maps


def _get_program():
    if "nc" not in _CACHE:
        _CACHE["nc"] = _build_program()
    return _CACHE["nc"]


def kernel(**inputs) -> np.ndarray:
    from concourse import bass_utils

    nc = _get_program()
    in_maps = _prep_inputs(inputs)
    res = bass_utils.run_bass_kernel_spmd(nc, in_maps, core_ids=list(range(NCORES)))
    _CACHE["last_results"] = res
    outs = np.stack([np.asarray(r["out"]) for r in res.results])    # [8, 512, V]
    full = outs.reshape(NCORES, NW, WSTEPS, BLOC, V).transpose(1, 2, 0, 3, 4)
    full = full.reshape(NW * WSTEPS, B, V)[:NSTEP]
    return np.ascontiguousarray(full.astype(np.float32))


# revision 4
# speedup vs baseline: 5919.8217x; 5919.8217x over previous
"""Trainium2 Bass kernel for nn_DecoderRNN (2-layer GRU decoder, teacher forcing).

Strategy (8 NeuronCores, no collectives):
  - Data-parallel over batch: each core runs the full 127-step GRU recurrence
    for its 4 of 32 sequences (B_loc=4).  Everything is kept in a transposed
    layout ([hidden-dim on partitions, batch on free]) so the gate math runs
    on all 128 partitions and the hidden state never needs transposing.
  - Input-side matmuls (embedding @ W_ih0, h0' @ W_ih1) are batched over
    32-step windows (M=128) for full PE utilisation; only the two h @ W_hh
    matmuls run per-step (weight-stationary, N=4, FWL bf16).
  - Output projection is row-sharded (each core projects its own 508 token
    rows over the full 32000 vocab) with a streaming online log-softmax, so
    log_softmax is fully local.  out_W^T is streamed from HBM in two passes
    (2 token-tiles per pass) so it is only read twice per core.

kernel(**inputs) takes the FULL unsharded inputs and returns the full
[127, 32, 32000] float32 log-prob tensor.
"""

import sys
import time

import numpy as np

try:
    import concourse  # noqa: F401
except ImportError:  # pragma: no cover
    sys.path.insert(0, "/opt/trn_rl_repo")

import ml_dtypes

import concourse.bass as bass
import concourse.tile as tile
from concourse import bacc, mybir
from concourse.bass import ds
from concourse.masks import make_identity

BF16 = mybir.dt.bfloat16
F32 = mybir.dt.float32
I32 = mybir.dt.int32
AF = mybir.ActivationFunctionType
PE = mybir.EngineType.PE

# Problem constants (hardcoded per contract).
V, E, H = 32000, 1024, 1024
B, T = 32, 128
NSTEP = T - 1          # 127 real recurrence steps
NCORES = 8
BLOC = B // NCORES     # 4 sequences per core
WSTEPS = 32            # recurrence steps per window
NW = 4                 # windows (128 steps incl. 1 pad step)
TOKW = WSTEPS * BLOC   # 128 local tokens per window
LTOK = NW * TOKW       # 512 local tokens (incl. pad)
G3 = 3 * H             # 3072 gate rows (r, z, n)
KC = H // 128          # 8 contraction chunks
MG = G3 // 128         # 24 gate-row groups of 128
HCW = WSTEPS * BLOC + BLOC   # 132 cols per h-chunk (4 carry + 32*4 state)
VCH = 500              # vocab chunk
NVC = V // VCH         # 64 vocab chunks
TT_PER_PASS = 2        # token-tiles sharing one out_W stream pass

_np_bf16 = ml_dtypes.bfloat16

_CACHE = {}


def _bf(x):
    return np.ascontiguousarray(np.asarray(x, dtype=np.float32).astype(_np_bf16))


def _wtiles(W):
    """[3H, H] weight -> SBUF lhsT layout [128, MG*KC*128] (bf16).

    sb[p, (m*KC+k)*128 + q] = W[128m + q, 128k + p]  (lhsT tile (m,k) = W-block^T)
    """
    W4 = np.asarray(W, dtype=np.float32).reshape(MG, 128, KC, 128)  # m q k p
    return _bf(W4.transpose(3, 0, 2, 1).reshape(128, MG * KC * 128))


def _build_program():
    nc = bacc.Bacc("TRN2", target_bir_lowering=False, debug=False,
                   num_devices=NCORES)

    # ---- DRAM I/O ----
    embW = nc.dram_tensor("embW", [V, E], F32, kind="ExternalInput")
    idx = nc.dram_tensor("idx", [LTOK, 1], I32, kind="ExternalInput")
    wih0 = nc.dram_tensor("wih0", [128, MG * KC * 128], BF16, kind="ExternalInput")
    whh0 = nc.dram_tensor("whh0", [128, MG * KC * 128], BF16, kind="ExternalInput")
    wih1 = nc.dram_tensor("wih1", [128, MG * KC * 128], BF16, kind="ExternalInput")
    whh1 = nc.dram_tensor("whh1", [128, MG * KC * 128], BF16, kind="ExternalInput")
    # biases: [1, G3] combined input-side bias per layer, [1, H] hidden n-bias
    bg0 = nc.dram_tensor("bg0", [1, G3], BF16, kind="ExternalInput")
    bn0 = nc.dram_tensor("bn0", [1, H], BF16, kind="ExternalInput")
    bg1 = nc.dram_tensor("bg1", [1, G3], BF16, kind="ExternalInput")
    bn1 = nc.dram_tensor("bn1", [1, H], BF16, kind="ExternalInput")
    hinit = nc.dram_tensor("hinit", [2, H, BLOC], BF16, kind="ExternalInput")
    woutT = nc.dram_tensor("woutT", [H, V], BF16, kind="ExternalInput")
    outb = nc.dram_tensor("outb", [1, V], BF16, kind="ExternalInput")
    out = nc.dram_tensor("out", [LTOK, V], F32, kind="ExternalOutput")

    with tile.TileContext(nc) as tc:
        with (
            tc.tile_pool(name="const", bufs=1) as cpool,
            tc.tile_pool(name="psum_t", bufs=2, space="PSUM") as tpsum,
        ):
            ident = cpool.tile([128, 128], F32)
            make_identity(nc, ident[:])
            ones = cpool.tile([1, 128], BF16)
            nc.gpsimd.memset(ones[:], 1.0)
            bg0_s = cpool.tile([1, G3], BF16)
            nc.sync.dma_start(bg0_s[:], bg0[:, :])
            bn0_s = cpool.tile([1, H], BF16)
            nc.sync.dma_start(bn0_s[:], bn0[:, :])
            bg1_s = cpool.tile([1, G3], BF16)
            nc.sync.dma_start(bg1_s[:], bg1[:, :])
            bn1_s = cpool.tile([1, H], BF16)
            nc.sync.dma_start(bn1_s[:], bn1[:, :])
            # h1 history for the projection: chunk c cols [512c : 512c+512]
            h1all = cpool.tile([128, KC * LTOK], BF16)

            # ================= GRU phase =================
            with (
                tc.tile_pool(name="gruw", bufs=1) as gw,
                tc.tile_pool(name="gwin", bufs=1) as gwin,
                tc.tile_pool(name="wstr", bufs=8) as wstr,
                tc.tile_pool(name="xemb", bufs=1) as xpool,
                tc.tile_pool(name="scr", bufs=2) as scr,
                tc.tile_pool(name="psum_b", bufs=2, space="PSUM") as bpsum,
                tc.tile_pool(name="psum_g", bufs=1, space="PSUM") as gpsum,
            ):
                whh0_s = gw.tile([128, MG * KC * 128], BF16, tag="whh0")
                nc.sync.dma_start(whh0_s[:], whh0[:, :])
                wih1_s = gw.tile([128, MG * KC * 128], BF16, tag="wih1")
                nc.sync.dma_start(wih1_s[:], wih1[:, :])
                whh1_s = gw.tile([128, MG * KC * 128], BF16, tag="whh1")
                nc.sync.dma_start(whh1_s[:], whh1[:, :])

                h0T = gwin.tile([128, KC * HCW], BF16, tag="h0T")
                h1T = gwin.tile([128, KC * HCW], BF16, tag="h1T")

                def sweep(wsb, giT, hT, bn_s):
                    """32 recurrence steps of one GRU layer (transposed layout)."""
                    ghp = gpsum.tile([128, MG * BLOC], F32, tag="ghp")
                    ghv = ghp[:].rearrange("p (m q) -> p m q", q=BLOC)
                    giv = giT[:].rearrange("p (m q) -> p m q", q=TOKW)
                    hv = hT[:].rearrange("p (c q) -> p c q", q=HCW)

                    def body(s):
                        for m in range(MG):
                            for k in range(KC):
                                nc.tensor.matmul(
                                    ghp[:, BLOC * m:BLOC * (m + 1)],
                                    lhsT=wsb[:, (m * KC + k) * 128:(m * KC + k + 1) * 128],
                                    rhs=hT[:, ds(k * HCW + s * BLOC, BLOC)],
                                    start=(k == 0),
                                    stop=(k == KC - 1 and m < 2 * KC))
                            if m >= 2 * KC:  # n-gate rows: add b_hh_n
                                nc.tensor.matmul(
                                    ghp[:, BLOC * m:BLOC * (m + 1)],
                                    lhsT=bn_s[0:1, 128 * (m - 2 * KC):128 * (m - 2 * KC + 1)],
                                    rhs=ones[0:1, 0:BLOC], start=False, stop=True)
                        # gates  (all [128, m-groups, 4] views)
                        rz = scr.tile([128, 16 * BLOC], F32, tag="rz")
                        rzv = rz[:].rearrange("p (m q) -> p m q", q=BLOC)
                        nc.vector.tensor_add(rzv[:, :, :], giv[:, 0:16, ds(s * BLOC, BLOC)],
                                             ghv[:, 0:16, :])
                        rza = scr.tile([128, 16 * BLOC], F32, tag="rza")
                        nc.scalar.activation(rza[:], rz[:], AF.Sigmoid)
                        rzav = rza[:].rearrange("p (m q) -> p m q", q=BLOC)
                        rn = scr.tile([128, 8 * BLOC], F32, tag="rn")
                        rnv = rn[:].rearrange("p (m q) -> p m q", q=BLOC)
                        nc.vector.tensor_mul(rnv[:, :, :], rzav[:, 0:8, :], ghv[:, 16:24, :])
                        t3 = scr.tile([128, 8 * BLOC], F32, tag="t3")
                        t3v = t3[:].rearrange("p (m q) -> p m q", q=BLOC)
                        nc.vector.tensor_add(t3v[:, :, :], rnv[:, :, :],
                                             giv[:, 16:24, ds(s * BLOC, BLOC)])
                        na = scr.tile([128, 8 * BLOC], F32, tag="na")
                        nc.scalar.activation(na[:], t3[:], AF.Tanh)
                        nav = na[:].rearrange("p (m q) -> p m q", q=BLOC)
                        d = scr.tile([128, 8 * BLOC], F32, tag="d")
                        dv = d[:].rearrange("p (m q) -> p m q", q=BLOC)
                        nc.vector.tensor_sub(dv[:, :, :], hv[:, :, ds(s * BLOC, BLOC)],
                                             nav[:, :, :])
                        zd = scr.tile([128, 8 * BLOC], F32, tag="zd")
                        zdv = zd[:].rearrange("p (m q) -> p m q", q=BLOC)
                        nc.vector.tensor_mul(zdv[:, :, :], rzav[:, 8:16, :], dv[:, :, :])
                        nc.vector.tensor_add(hv[:, :, ds(s * BLOC + BLOC, BLOC)],
                                             nav[:, :, :], zdv[:, :, :])

                    tc.For_i_unrolled_general(
                        start=0, end=WSTEPS, step=1,
                        unrollable_body=lambda iv, u: [body(iv + i) for i in range(u)],
                        max_unroll=4, hint_engines=(PE,))

                for w in range(NW):
                    # ---- gather + transpose this window's embeddings ----
                    idxt = xpool.tile([128, 1], I32, tag="idxt")
                    nc.sync.dma_start(idxt[:], idx[TOKW * w:TOKW * (w + 1), :])
                    xg = xpool.tile([128, E], F32, tag="xg")
                    nc.gpsimd.indirect_dma_start(
                        out=xg[:], out_offset=None, in_=embW[:, :],
                        in_offset=bass.IndirectOffsetOnAxis(ap=idxt[:, 0:1], axis=0))
                    xT = xpool.tile([128, KC * 128], BF16, tag="xT")
                    for c in range(KC):
                        pt = tpsum.tile([128, 128], F32, tag="pt")
                        nc.tensor.transpose(out=pt[:], in_=xg[:, 128 * c:128 * (c + 1)],
                                            identity=ident[:])
                        nc.scalar.copy(out=xT[:, 128 * c:128 * (c + 1)], in_=pt[:])
                    # ---- carry-in hidden state ----
                    for l, hT_l in ((0, h0T), (1, h1T)):
                        hv_l = hT_l[:].rearrange("p (c q) -> p c q", q=HCW)
                        if w == 0:
                            nc.sync.dma_start(
                                hv_l[:, :, 0:BLOC],
                                hinit[l, :, :].rearrange("(c p) q -> p c q", p=128))
                        else:
                            nc.vector.tensor_copy(
                                hv_l[:, :, 0:BLOC],
                                hv_l[:, :, WSTEPS * BLOC:WSTEPS * BLOC + BLOC])
                    # ---- layer 0 ----
                    giT = gwin.tile([128, MG * TOKW], BF16, tag="giT")
                    for m in range(MG):
                        pb = bpsum.tile([128, 128], F32, tag="pb")
                        for k in range(KC):
                            wt = wstr.tile([128, 128], BF16, tag="wt")
                            nc.sync.dma_start(
                                wt[:], wih0[:, (m * KC + k) * 128:(m * KC + k + 1) * 128])
                            nc.tensor.matmul(pb[:], lhsT=wt[:],
                                             rhs=xT[:, 128 * k:128 * (k + 1)],
                                             start=(k == 0), stop=False)
                        nc.tensor.matmul(pb[:], lhsT=bg0_s[0:1, 128 * m:128 * (m + 1)],
                                         rhs=ones[0:1, 0:128], start=False, stop=True)
                        nc.scalar.copy(out=giT[:, 128 * m:128 * (m + 1)], in_=pb[:])
                    sweep(whh0_s, giT, h0T, bn0_s)
                    # ---- layer 1 ----
                    giT = gwin.tile([128, MG * TOKW], BF16, tag="giT")
                    for m in range(MG):
                        pb = bpsum.tile([128, 128], F32, tag="pb")
                        for k in range(KC):
                            nc.tensor.matmul(
                                pb[:],
                                lhsT=wih1_s[:, (m * KC + k) * 128:(m * KC + k + 1) * 128],
                                rhs=h0T[:, k * HCW + BLOC:k * HCW + BLOC + TOKW],
                                start=(k == 0), stop=False)
                        nc.tensor.matmul(pb[:], lhsT=bg1_s[0:1, 128 * m:128 * (m + 1)],
                                         rhs=ones[0:1, 0:128], start=False, stop=True)
                        nc.scalar.copy(out=giT[:, 128 * m:128 * (m + 1)], in_=pb[:])
                    sweep(whh1_s, giT, h1T, bn1_s)
                    # ---- stash h1 history for the projection ----
                    for c in range(KC):
                        nc.scalar.copy(
                            out=h1all[:, c * LTOK + TOKW * w:c * LTOK + TOKW * (w + 1)],
                            in_=h1T[:, c * HCW + BLOC:c * HCW + BLOC + TOKW])

            # ================= projection phase =================
            with (
                tc.tile_pool(name="wout", bufs=6) as wpool,
                tc.tile_pool(name="bstr", bufs=2) as bpool,
                tc.tile_pool(name="xs", bufs=1) as xspool,
                tc.tile_pool(name="stat", bufs=2) as spool,
                tc.tile_pool(name="ostg", bufs=4) as opool,
                tc.tile_pool(name="psum_p", bufs=2, space="PSUM") as ppsum,
            ):
                for half in range(LTOK // TOKW // TT_PER_PASS):
                    tts = [half * TT_PER_PASS + i for i in range(TT_PER_PASS)]
                    xs = {t: xspool.tile([128, V], BF16, tag=f"xs{t % TT_PER_PASS}", name=f"xs{t}")
                          for t in tts}
                    m_arr = {t: spool.tile([128, NVC], F32, tag=f"ma{t % TT_PER_PASS}", name=f"ma{t}")
                             for t in tts}
                    s_arr = {t: spool.tile([128, NVC], F32, tag=f"sa{t % TT_PER_PASS}", name=f"sa{t}")
                             for t in tts}
                    for v in range(NVC):
                        wts = []
                        for k in range(KC):
                            wt = wpool.tile([128, VCH], BF16, tag="wt", name="wtp")
                            nc.sync.dma_start(
                                wt[:], woutT[128 * k:128 * (k + 1), VCH * v:VCH * (v + 1)])
                            wts.append(wt)
                        bt = bpool.tile([1, VCH], BF16, tag="bt")
                        nc.sync.dma_start(bt[:], outb[0:1, VCH * v:VCH * (v + 1)])
                        for t in tts:
                            pp = ppsum.tile([128, VCH], F32, tag="pp")
                            for k in range(KC):
                                nc.tensor.matmul(
                                    pp[:],
                                    lhsT=h1all[:, k * LTOK + TOKW * t:k * LTOK + TOKW * (t + 1)],
                                    rhs=wts[k][:], start=(k == 0), stop=False)
                            nc.tensor.matmul(pp[:], lhsT=ones[0:1, 0:128], rhs=bt[0:1, :],
                                             start=False, stop=True)
                            nc.vector.reduce_max(m_arr[t][:, v:v + 1], pp[:],
                                                 axis=mybir.AxisListType.X)
                            negm = spool.tile([128, 1], F32, tag="negm")
                            nc.vector.tensor_scalar_mul(negm[:], m_arr[t][:, v:v + 1], -1.0)
                            nc.scalar.activation(xs[t][:, VCH * v:VCH * (v + 1)], pp[:],
                                                 AF.Identity, bias=negm[:])
                            es = spool.tile([128, VCH], F32, tag="es")
                            nc.scalar.activation(es[:], pp[:], AF.Exp, bias=negm[:],
                                                 accum_out=s_arr[t][:, v:v + 1])
                    for t in tts:
                        M = spool.tile([128, 1], F32, tag="M")
                        nc.vector.reduce_max(M[:], m_arr[t][:], axis=mybir.AxisListType.X)
                        negM = spool.tile([128, 1], F32, tag="negM")
                        nc.vector.tensor_scalar_mul(negM[:], M[:], -1.0)
                        ee = spool.tile([128, NVC], F32, tag="ee")
                        nc.scalar.activation(ee[:], m_arr[t][:], AF.Exp, bias=negM[:])
                        se = spool.tile([128, NVC], F32, tag="se")
                        nc.vector.tensor_mul(se[:], s_arr[t][:], ee[:])
                        sf = spool.tile([128, 1], F32, tag="sf")
                        nc.vector.reduce_sum(sf[:], se[:], axis=mybir.AxisListType.X)
                        lse = spool.tile([128, 1], F32, tag="lse")
                        nc.scalar.activation(lse[:], sf[:], AF.Ln)
                        nc.vector.tensor_add(lse[:], lse[:], M[:])
                        delta = spool.tile([128, NVC], F32, tag="delta")
                        nc.vector.tensor_scalar_sub(delta[:], m_arr[t][:], lse[:])
                        for v in range(NVC):
                            og = opool.tile([128, VCH], F32, tag="og")
                            nc.scalar.activation(og[:], xs[t][:, VCH * v:VCH * (v + 1)],
                                                 AF.Identity, bias=delta[:, v:v + 1])
                            nc.sync.dma_start(
                                out[TOKW * t:TOKW * (t + 1), VCH * v:VCH * (v + 1)], og[:])

    nc.compile()
    return nc


def _prep_inputs(inputs):
    target = np.asarray(inputs["target"]).astype(np.int32)          # [B, T]
    enc = np.asarray(inputs["encoder_hidden"], dtype=np.float32)    # [2, B, H]
    emb_W = np.ascontiguousarray(np.asarray(inputs["emb_W"], dtype=np.float32))
    W_ih = np.asarray(inputs["W_ih"], dtype=np.float32)             # [2, 3H, E]
    W_hh = np.asarray(inputs["W_hh"], dtype=np.float32)             # [2, 3H, H]
    b_ih = np.asarray(inputs["b_ih"], dtype=np.float32)             # [2, 3H]
    b_hh = np.asarray(inputs["b_hh"], dtype=np.float32)             # [2, 3H]
    out_W = np.asarray(inputs["out_W"], dtype=np.float32)           # [V, H]
    out_b = np.asarray(inputs["out_b"], dtype=np.float32)           # [V]

    shared = {
        "embW": emb_W,
        "wih0": _wtiles(W_ih[0]), "whh0": _wtiles(W_hh[0]),
        "wih1": _wtiles(W_ih[1]), "whh1": _wtiles(W_hh[1]),
        "woutT": _bf(np.ascontiguousarray(out_W.T)),
        "outb": _bf(out_b.reshape(1, V)),
    }
    for l, (nm_g, nm_n) in enumerate((("bg0", "bn0"), ("bg1", "bn1"))):
        bg = b_ih[l].copy()
        bg[:2 * H] += b_hh[l][:2 * H]
        shared[nm_g] = _bf(bg.reshape(1, G3))
        shared[nm_n] = _bf(b_hh[l][2 * H:].reshape(1, H))

    in_maps = []
    tpad = np.minimum(np.arange(NW * WSTEPS), NSTEP - 1)  # step 127 -> pad
    valid = np.arange(NW * WSTEPS) < NSTEP
    for k in range(NCORES):
        rows = target[BLOC * k:BLOC * (k + 1)]            # [4, T]
        idx = np.where(valid[:, None], rows.T[tpad], 0)   # [128, 4]
        m = dict(shared)
        m["idx"] = np.ascontiguousarray(idx.reshape(LTOK, 1).astype(np.int32))
        m["hinit"] = _bf(enc[:, BLOC * k:BLOC * (k + 1), :].transpose(0, 2, 1))
        in_maps.append(m)
    return in_maps


def _get_program():
    if "nc" not in _CACHE:
        _CACHE["nc"] = _build_program()
    return _CACHE["nc"]


def _install_ntff_hook():
    """Make run_bass_kernel_spmd(trace=True) work under axon in this image:
    register the ctypes NTFF profile hook that the boot path skips when
    antenv.axon_hooks is absent, and stub out the artifact bucket upload."""
    import types

    if _CACHE.get("hook_done"):
        return
    _CACHE["hook_done"] = True
    try:
        import antenv
        from concourse import bass_utils

        bass_utils.upload_artifacts = lambda tmpdir: tmpdir  # no bucket in container
        if "antenv.axon_hooks" not in sys.modules:
            mod = types.ModuleType("antenv.axon_hooks")
            state = {}
            mod.set_axon_ntff_profile_hook = lambda h: state.__setitem__("h", h)
            mod.get_axon_ntff_profile_hook = lambda: state.get("h")
            sys.modules["antenv.axon_hooks"] = mod
            antenv.axon_hooks = mod
        if "/root/.axon_site" not in sys.path:
            sys.path.insert(0, "/root/.axon_site")
        from antenv.axon_hooks import set_axon_ntff_profile_hook
        from trn_agent_boot.trn_boot import _ntff_profile_via_ctypes

        set_axon_ntff_profile_hook(
            _ntff_profile_via_ctypes("/opt/axon/libaxon_pjrt.so"))
    except Exception as exc:  # pragma: no cover
        print(f"ntff hook install failed ({exc}); tracing unavailable")


def kernel(**inputs) -> np.ndarray:
    from concourse import bass_utils

    trace = bool(_CACHE.get("trace"))
    if trace:
        _install_ntff_hook()
    nc = _get_program()
    in_maps = _prep_inputs(inputs)
    res = bass_utils.run_bass_kernel_spmd(nc, in_maps, core_ids=list(range(NCORES)),
                                          trace=trace)
    _CACHE["last_results"] = res
    outs = np.stack([np.asarray(r["out"]) for r in res.results])    # [8, 512, V]
    full = outs.reshape(NCORES, NW, WSTEPS, BLOC, V).transpose(1, 2, 0, 3, 4)
    full = full.reshape(NW * WSTEPS, B, V)[:NSTEP]
    return np.ascontiguousarray(full.astype(np.float32))


# revision 8
# speedup vs baseline: 7872.0504x; 1.3298x over previous
"""Trainium2 Bass kernel for nn_DecoderRNN (2-layer GRU decoder, teacher forcing).

Strategy (8 NeuronCores, no collectives):
  - Data-parallel over batch: each core runs the full 127-step GRU recurrence
    for its 4 of 32 sequences (B_loc=4).
  - Recurrent matmuls are batch-major: hidden state h^T is the stationary
    operand (M=4, LDWEIGHTS ~free) and the GRU weights stream as the moving
    operand in N=512 tiles, keeping PE duty high (HAM stays un-throttled).
    Recurrent weights are fp8e4m3 (x16 pre-scaled, rescaled in the gate math)
    so all three stay resident in SBUF.
  - Input-side matmuls (embedding @ W_ih0 per window, h0' @ W_ih1 per 4-step
    sub-window) are batched for weight-stream amortisation; the two layers
    interleave at sub-window granularity so PE never idles on the gate chain.
  - h' is re-transposed each step via PE-transpose (8x [4,128] tiles) into the
    per-window h^T history, which doubles as next-step stationary operands,
    the W_ih1 batch input, and the projection's lhsT.
  - Output projection is row-sharded (each core projects its own 508 token
    rows over the full 32000 vocab) with a streaming online log-softmax
    (fully local, no collectives).  out_W^T bf16 streams from HBM in two
    passes (2 token-tiles per pass).

kernel(**inputs) takes the FULL unsharded inputs and returns the full
[127, 32, 32000] float32 log-prob tensor.
"""

import sys

import numpy as np

try:
    import concourse  # noqa: F401
except ImportError:  # pragma: no cover
    sys.path.insert(0, "/opt/trn_rl_repo")

import ml_dtypes

import concourse.bass as bass
import concourse.tile as tile
from concourse import bacc, mybir
from concourse.masks import make_identity

BF16 = mybir.dt.bfloat16
FP8 = mybir.dt.float8e4
F32 = mybir.dt.float32
I32 = mybir.dt.int32
AF = mybir.ActivationFunctionType
ALU = mybir.AluOpType

# Problem constants (hardcoded per contract).
V, E, H = 32000, 1024, 1024
B, T = 32, 128
NSTEP = T - 1          # 127 real recurrence steps
NCORES = 8
BLOC = B // NCORES     # 4 sequences per core
WSTEPS = 32            # recurrence steps per window
NW = 4                 # windows (128 steps incl. 1 pad step)
SUB = 4                # layer-interleave granularity (steps)
TOKW = WSTEPS * BLOC   # 128 local tokens per window
LTOK = NW * TOKW       # 512 local tokens (incl. pad)
G3 = 3 * H             # 3072 gate rows (r, z, n)
KC = H // 128          # 8 contraction chunks
NT = G3 // 512         # 6 N-tiles of 512 in the gate dim
HCW = WSTEPS * BLOC + BLOC   # 132 cols per h-chunk (4 carry + 32*4 state)
VCH = 500              # vocab chunk
NVC = V // VCH         # 64 vocab chunks
TT_PER_PASS = 2        # token-tiles sharing one out_W stream pass
W8SCALE = 16.0         # fp8 weight pre-scale (undone in gate math)

_np_bf16 = ml_dtypes.bfloat16
_np_fp8 = ml_dtypes.float8_e4m3

_CACHE = {}


def _bf(x):
    return np.ascontiguousarray(np.asarray(x, dtype=np.float32).astype(_np_bf16))


def _wmoving(W, dtype):
    """[3H, H] weight -> moving-operand layout [128, KC*G3].

    sb[p, k*G3 + c] = W[c, 128k + p]   (rhs tile (k, n) = W^T block)
    """
    WT = np.asarray(W, dtype=np.float32).T.reshape(KC, 128, G3)
    out = np.ascontiguousarray(WT.transpose(1, 0, 2).reshape(128, KC * G3))
    return np.ascontiguousarray(out.astype(dtype))


def _build_program():
    nc = bacc.Bacc("TRN2", target_bir_lowering=False, debug=False,
                   num_devices=NCORES)

    # ---- DRAM I/O ----
    embW = nc.dram_tensor("embW", [V, E], F32, kind="ExternalInput")
    idx = nc.dram_tensor("idx", [LTOK, 1], I32, kind="ExternalInput")
    wih0 = nc.dram_tensor("wih0", [128, KC * G3], BF16, kind="ExternalInput")
    whh0 = nc.dram_tensor("whh0", [128, KC * G3], FP8, kind="ExternalInput")
    wih1 = nc.dram_tensor("wih1", [128, KC * G3], BF16, kind="ExternalInput")
    whh1 = nc.dram_tensor("whh1", [128, KC * G3], FP8, kind="ExternalInput")
    # combined input-side bias per layer [1, G3]; x16-scaled hidden n-bias [1, H]
    bg0 = nc.dram_tensor("bg0", [1, G3], BF16, kind="ExternalInput")
    bn0 = nc.dram_tensor("bn0", [1, H], BF16, kind="ExternalInput")
    bg1 = nc.dram_tensor("bg1", [1, G3], BF16, kind="ExternalInput")
    bn1 = nc.dram_tensor("bn1", [1, H], BF16, kind="ExternalInput")
    hinit = nc.dram_tensor("hinit", [2, H, BLOC], BF16, kind="ExternalInput")
    hinit_bm = nc.dram_tensor("hinit_bm", [2, BLOC, H], BF16, kind="ExternalInput")
    woutT = nc.dram_tensor("woutT", [H, V], BF16, kind="ExternalInput")
    outb = nc.dram_tensor("outb", [1, V], BF16, kind="ExternalInput")
    out = nc.dram_tensor("out", [LTOK, V], F32, kind="ExternalOutput")

    RS = 1.0 / W8SCALE

    with tile.TileContext(nc) as tc:
        with (
            tc.tile_pool(name="const", bufs=1) as cpool,
            tc.tile_pool(name="dramp", bufs=1, space="DRAM") as dpool,
        ):
            ident = cpool.tile([128, 128], BF16)
            make_identity(nc, ident[:])
            ones = cpool.tile([1, 128], BF16)
            nc.gpsimd.memset(ones[:], 1.0)
            bg0_s = cpool.tile([1, G3], BF16)
            nc.sync.dma_start(bg0_s[:], bg0[:, :])
            bn0_s = cpool.tile([1, H], BF16)
            nc.sync.dma_start(bn0_s[:], bn0[:, :])
            bg1_s = cpool.tile([1, G3], BF16)
            nc.sync.dma_start(bg1_s[:], bg1[:, :])
            bn1_s = cpool.tile([1, H], BF16)
            nc.sync.dma_start(bn1_s[:], bn1[:, :])
            # h1 history for the projection, staged via DRAM (SBUF is tight
            # during the GRU phase): chunk c, token t at [c, t]
            h1hist = dpool.tile([128, KC * LTOK], BF16)

            # ================= GRU phase =================
            with (
                tc.tile_pool(name="gruw", bufs=1) as gw,
                tc.tile_pool(name="gwin", bufs=1) as gwin,
                tc.tile_pool(name="wstr", bufs=6) as wstr,
                tc.tile_pool(name="xemb", bufs=1) as xpool,
                tc.tile_pool(name="scr", bufs=1) as scr,
                tc.tile_pool(name="hbm", bufs=3) as hbmp,
                tc.tile_pool(name="gis", bufs=2) as gisp,
                tc.tile_pool(name="g1p", bufs=2) as g1p,
                tc.tile_pool(name="psum_gh", bufs=1, space="PSUM") as ghpsum,
                tc.tile_pool(name="psum_tp", bufs=2, space="PSUM") as ptp,
            ):
                whh0_s = gw.tile([128, KC * G3], FP8, tag="whh0")
                nc.sync.dma_start(whh0_s[:], whh0[:, :])
                wih1_s = gw.tile([128, KC * G3], BF16, tag="wih1")
                nc.sync.dma_start(wih1_s[:], wih1[:, :])
                whh1_s = gw.tile([128, KC * G3], FP8, tag="whh1")
                nc.sync.dma_start(whh1_s[:], whh1[:, :])

                # transposed state history per layer: chunk c col 4(s+1) = h'(s)
                xh = [gwin.tile([128, KC * HCW], BF16, tag="xh0", name="xh0"),
                      gwin.tile([128, KC * HCW], BF16, tag="xh1", name="xh1")]
                # batch-major gi buffers: full window for layer 0, one
                # sub-window for layer 1 (rotated), plus per-step base-0
                # staging tiles (engines need 32-aligned partition bases;
                # DMA does the unaligned row extraction).
                gi0_sb = gwin.tile([128, G3], BF16, tag="gi0", name="gi0")

                prev_hbm = [None, None]   # batch-major h chain per layer
                cur_gi1 = [None]          # current sub-window gi1 tile

                def step(l, w, s):
                    """One recurrence step of layer l (batch-major gh)."""
                    wsb = whh0_s if l == 0 else whh1_s
                    bn_s = bn0_s if l == 0 else bn1_s
                    xh_l = xh[l]
                    # stage this step's gi rows at partition base 0 via DMA
                    gis = gisp.tile([BLOC, G3], BF16, tag=f"gis{l}", name=f"gis{l}")
                    if l == 0:
                        nc.sync.dma_start(gis[:], gi0_sb[BLOC * s:BLOC * (s + 1), :])
                    else:
                        j = s % SUB
                        nc.sync.dma_start(
                            gis[:], cur_gi1[0][BLOC * j:BLOC * (j + 1), :])
                    # --- gh = h @ W_hh^T (+16*b_hh_n on the n third) ---
                    ph = ghpsum.tile([BLOC, G3], F32, tag="ph", name="ph")
                    for k in range(KC):
                        lt = xh_l[:, k * HCW + SUB * s:k * HCW + SUB * s + BLOC]
                        for n in range(NT):
                            nc.tensor.matmul(
                                ph[:, 512 * n:512 * (n + 1)], lhsT=lt,
                                rhs=wsb[:, k * G3 + 512 * n:k * G3 + 512 * (n + 1)],
                                start=(k == 0), stop=(k == KC - 1 and n < 4))
                    for n in range(2):
                        nc.tensor.matmul(
                            ph[:, 2 * H + 512 * n:2 * H + 512 * (n + 1)],
                            lhsT=ones[0:1, 0:BLOC],
                            rhs=bn_s[0:1, 512 * n:512 * (n + 1)],
                            start=False, stop=True)
                    # --- gates (batch-major, partitions 0:4) ---
                    G = gis[:]
                    srz = scr.tile([BLOC, 2 * H], F32, tag="srz", name="srz")
                    nc.vector.scalar_tensor_tensor(
                        srz[:], ph[:, 0:2 * H], RS, G[:, 0:2 * H],
                        op0=ALU.mult, op1=ALU.add)
                    nc.scalar.activation(srz[:], srz[:], AF.Sigmoid)
                    rn = scr.tile([BLOC, H], F32, tag="rn", name="rn")
                    nc.vector.scalar_tensor_tensor(
                        rn[:], ph[:, 2 * H:G3], RS, srz[:, 0:H],
                        op0=ALU.mult, op1=ALU.mult)
                    nc.vector.tensor_add(rn[:], rn[:], G[:, 2 * H:G3])
                    na = scr.tile([BLOC, H], F32, tag="na", name="na")
                    nc.scalar.activation(na[:], rn[:], AF.Tanh)
                    d = scr.tile([BLOC, H], F32, tag="d", name="d")
                    nc.vector.tensor_sub(d[:], prev_hbm[l][:], na[:])
                    nc.vector.tensor_mul(d[:], srz[:, H:2 * H], d[:])
                    hb = hbmp.tile([BLOC, H], BF16, tag=f"hb{l}", name=f"hb{l}")
                    nc.vector.tensor_add(hb[:], na[:], d[:])
                    prev_hbm[l] = hb
                    # --- h'^T back into the history (8x PE transpose) ---
                    for c in range(KC):
                        pt = ptp.tile([128, BLOC], BF16, tag="pt", name="pt")
                        nc.tensor.transpose(
                            out=pt[:], in_=hb[:, 128 * c:128 * (c + 1)],
                            identity=ident[0:BLOC, 0:BLOC])
                        nc.scalar.copy(
                            out=xh_l[:, k_off(c) + SUB * (s + 1):
                                     k_off(c) + SUB * (s + 1) + BLOC],
                            in_=pt[:])

                def k_off(c):
                    return c * HCW

                def gi_batch(l, dst_rows, lts, bias_s, wsrc_sb, wsrc_dram):
                    """Batched input-side matmul: dst = x @ W_ih^T + b (bf16)."""
                    for n in range(NT):
                        pb = ptp.tile([128, 512], F32, tag="pt", name="pb")
                        mrows = lts[0].shape[-1]
                        for k in range(KC):
                            if wsrc_dram is not None:
                                wt = wstr.tile([128, 512], BF16, tag="wt", name="wt")
                                nc.sync.dma_start(
                                    wt[:],
                                    wsrc_dram[:, k * G3 + 512 * n:k * G3 + 512 * (n + 1)])
                                rhs = wt[:]
                            else:
                                rhs = wsrc_sb[:, k * G3 + 512 * n:k * G3 + 512 * (n + 1)]
                            nc.tensor.matmul(pb[0:mrows, :], lhsT=lts[k], rhs=rhs,
                                             start=(k == 0), stop=False)
                        nc.tensor.matmul(pb[0:mrows, :], lhsT=ones[0:1, 0:mrows],
                                         rhs=bias_s[0:1, 512 * n:512 * (n + 1)],
                                         start=False, stop=True)
                        nc.scalar.copy(out=dst_rows[:, 512 * n:512 * (n + 1)],
                                       in_=pb[0:mrows, :])

                for w in range(NW):
                    # ---- gather + transpose this window's embeddings ----
                    idxt = xpool.tile([128, 1], I32, tag="idxt", name="idxt")
                    nc.sync.dma_start(idxt[:], idx[TOKW * w:TOKW * (w + 1), :])
                    xgb = xpool.tile([128, E], BF16, tag="xgb", name="xgb")
                    xg = xpool.tile([128, E], F32, tag="xg", name="xg")
                    nc.gpsimd.indirect_dma_start(
                        out=xg[:], out_offset=None, in_=embW[:, :],
                        in_offset=bass.IndirectOffsetOnAxis(ap=idxt[:, 0:1], axis=0))
                    nc.scalar.copy(out=xgb[:], in_=xg[:])
                    xT = xpool.tile([128, KC * 128], BF16, tag="xT", name="xT")
                    for c in range(KC):
                        pt = ptp.tile([128, 128], BF16, tag="pt", name="pt")
                        nc.tensor.transpose(out=pt[:],
                                            in_=xgb[:, 128 * c:128 * (c + 1)],
                                            identity=ident[:])
                        nc.scalar.copy(out=xT[:, 128 * c:128 * (c + 1)], in_=pt[:])
                    # ---- carry-in hidden state ----
                    for l in (0, 1):
                        hv_l = xh[l][:].rearrange("p (c q) -> p c q", q=HCW)
                        if w == 0:
                            nc.sync.dma_start(
                                hv_l[:, :, 0:BLOC],
                                hinit[l, :, :].rearrange("(c p) q -> p c q", p=128))
                            hb0 = hbmp.tile([BLOC, H], BF16, tag=f"hb{l}",
                                            name=f"hbi{l}")
                            nc.sync.dma_start(hb0[:], hinit_bm[l, :, :])
                            prev_hbm[l] = hb0
                        else:
                            nc.vector.tensor_copy(
                                hv_l[:, :, 0:BLOC],
                                hv_l[:, :, WSTEPS * BLOC:WSTEPS * BLOC + BLOC])
                    # ---- layer-0 input batch (whole window) ----
                    gi_batch(0, gi0_sb[:, :],
                             [xT[:, 128 * k:128 * (k + 1)] for k in range(KC)],
                             bg0_s, None, wih0)
                    # ---- interleaved sweeps, SUB steps at a time ----
                    for sub in range(WSTEPS // SUB):
                        for j in range(SUB):
                            step(0, w, sub * SUB + j)
                        g1 = g1p.tile([SUB * BLOC, G3], BF16, tag="g1", name="g1")
                        cur_gi1[0] = g1
                        gi_batch(1, g1[:, :],
                                 [xh[0][:, c * HCW + SUB * BLOC * sub + BLOC:
                                        c * HCW + SUB * BLOC * (sub + 1) + BLOC]
                                  for c in range(KC)],
                                 bg1_s, wih1_s, None)
                        for j in range(SUB):
                            step(1, w, sub * SUB + j)
                    # ---- stash h1 history for the projection ----
                    for c in range(KC):
                        nc.sync.dma_start(
                            h1hist[:, c * LTOK + TOKW * w:c * LTOK + TOKW * (w + 1)],
                            xh[1][:, c * HCW + BLOC:c * HCW + BLOC + TOKW])

            # ================= projection phase =================
            with (
                tc.tile_pool(name="h1p", bufs=1) as h1p,
                tc.tile_pool(name="wout", bufs=6) as wpool,
                tc.tile_pool(name="bstr", bufs=2) as bpool,
                tc.tile_pool(name="xs", bufs=1) as xspool,
                tc.tile_pool(name="stat", bufs=2) as spool,
                tc.tile_pool(name="ostg", bufs=4) as opool,
                tc.tile_pool(name="psum_p", bufs=2, space="PSUM") as ppsum,
            ):
                h1all = h1p.tile([128, KC * LTOK], BF16)
                nc.sync.dma_start(h1all[:], h1hist[:])
                for half in range(LTOK // TOKW // TT_PER_PASS):
                    tts = [half * TT_PER_PASS + i for i in range(TT_PER_PASS)]
                    xs = {t: xspool.tile([128, V], BF16, tag=f"xs{t % TT_PER_PASS}",
                                         name=f"xs{t}") for t in tts}
                    m_arr = {t: spool.tile([128, NVC], F32, tag=f"ma{t % TT_PER_PASS}",
                                           name=f"ma{t}") for t in tts}
                    s_arr = {t: spool.tile([128, NVC], F32, tag=f"sa{t % TT_PER_PASS}",
                                           name=f"sa{t}") for t in tts}
                    for v in range(NVC):
                        wts = []
                        for k in range(KC):
                            wt = wpool.tile([128, VCH], BF16, tag="wt", name="wtp")
                            nc.sync.dma_start(
                                wt[:], woutT[128 * k:128 * (k + 1), VCH * v:VCH * (v + 1)])
                            wts.append(wt)
                        bt = bpool.tile([1, VCH], BF16, tag="bt", name="bt")
                        nc.sync.dma_start(bt[:], outb[0:1, VCH * v:VCH * (v + 1)])
                        for t in tts:
                            pp = ppsum.tile([128, VCH], F32, tag="pp", name="pp")
                            for k in range(KC):
                                nc.tensor.matmul(
                                    pp[:],
                                    lhsT=h1all[:, k * LTOK + TOKW * t:k * LTOK + TOKW * (t + 1)],
                                    rhs=wts[k][:], start=(k == 0), stop=False)
                            nc.tensor.matmul(pp[:], lhsT=ones[0:1, 0:128], rhs=bt[0:1, :],
                                             start=False, stop=True)
                            nc.vector.reduce_max(m_arr[t][:, v:v + 1], pp[:],
                                                 axis=mybir.AxisListType.X)
                            negm = spool.tile([128, 1], F32, tag="negm", name="negm")
                            nc.vector.tensor_scalar_mul(negm[:], m_arr[t][:, v:v + 1], -1.0)
                            nc.scalar.activation(xs[t][:, VCH * v:VCH * (v + 1)], pp[:],
                                                 AF.Identity, bias=negm[:])
                            es = spool.tile([128, VCH], F32, tag="es", name="es")
                            nc.scalar.activation(es[:], pp[:], AF.Exp, bias=negm[:],
                                                 accum_out=s_arr[t][:, v:v + 1])
                    for t in tts:
                        M = spool.tile([128, 1], F32, tag="M", name="M")
                        nc.vector.reduce_max(M[:], m_arr[t][:], axis=mybir.AxisListType.X)
                        negM = spool.tile([128, 1], F32, tag="negM", name="negM")
                        nc.vector.tensor_scalar_mul(negM[:], M[:], -1.0)
                        ee = spool.tile([128, NVC], F32, tag="ee", name="ee")
                        nc.scalar.activation(ee[:], m_arr[t][:], AF.Exp, bias=negM[:])
                        se = spool.tile([128, NVC], F32, tag="se", name="se")
                        nc.vector.tensor_mul(se[:], s_arr[t][:], ee[:])
                        sf = spool.tile([128, 1], F32, tag="sf", name="sf")
                        nc.vector.reduce_sum(sf[:], se[:], axis=mybir.AxisListType.X)
                        lse = spool.tile([128, 1], F32, tag="lse", name="lse")
                        nc.scalar.activation(lse[:], sf[:], AF.Ln)
                        nc.vector.tensor_add(lse[:], lse[:], M[:])
                        delta = spool.tile([128, NVC], F32, tag="delta", name="delta")
                        nc.vector.tensor_scalar_sub(delta[:], m_arr[t][:], lse[:])
                        for v in range(NVC):
                            og = opool.tile([128, VCH], F32, tag="og", name="og")
                            nc.scalar.activation(og[:], xs[t][:, VCH * v:VCH * (v + 1)],
                                                 AF.Identity, bias=delta[:, v:v + 1])
                            nc.sync.dma_start(
                                out[TOKW * t:TOKW * (t + 1), VCH * v:VCH * (v + 1)], og[:])

    nc.compile()
    return nc


def _prep_inputs(inputs):
    target = np.asarray(inputs["target"]).astype(np.int32)          # [B, T]
    enc = np.asarray(inputs["encoder_hidden"], dtype=np.float32)    # [2, B, H]
    emb_W = np.ascontiguousarray(np.asarray(inputs["emb_W"], dtype=np.float32))
    W_ih = np.asarray(inputs["W_ih"], dtype=np.float32)             # [2, 3H, E]
    W_hh = np.asarray(inputs["W_hh"], dtype=np.float32)             # [2, 3H, H]
    b_ih = np.asarray(inputs["b_ih"], dtype=np.float32)             # [2, 3H]
    b_hh = np.asarray(inputs["b_hh"], dtype=np.float32)             # [2, 3H]
    out_W = np.asarray(inputs["out_W"], dtype=np.float32)           # [V, H]
    out_b = np.asarray(inputs["out_b"], dtype=np.float32)           # [V]

    shared = {
        "embW": emb_W,
        "wih0": _wmoving(W_ih[0], _np_bf16),
        "whh0": _wmoving(W_hh[0] * W8SCALE, _np_fp8),
        "wih1": _wmoving(W_ih[1], _np_bf16),
        "whh1": _wmoving(W_hh[1] * W8SCALE, _np_fp8),
        "woutT": _bf(np.ascontiguousarray(out_W.T)),
        "outb": _bf(out_b.reshape(1, V)),
    }
    for l, (nm_g, nm_n) in enumerate((("bg0", "bn0"), ("bg1", "bn1"))):
        bg = b_ih[l].copy()
        bg[:2 * H] += b_hh[l][:2 * H]
        shared[nm_g] = _bf(bg.reshape(1, G3))
        shared[nm_n] = _bf(b_hh[l][2 * H:].reshape(1, H) * W8SCALE)

    in_maps = []
    tpad = np.minimum(np.arange(NW * WSTEPS), NSTEP - 1)  # step 127 -> pad
    valid = np.arange(NW * WSTEPS) < NSTEP
    for k in range(NCORES):
        rows = target[BLOC * k:BLOC * (k + 1)]            # [4, T]
        idx = np.where(valid[:, None], rows.T[tpad], 0)   # [128, 4]
        m = dict(shared)
        m["idx"] = np.ascontiguousarray(idx.reshape(LTOK, 1).astype(np.int32))
        m["hinit"] = _bf(enc[:, BLOC * k:BLOC * (k + 1), :].transpose(0, 2, 1))
        m["hinit_bm"] = _bf(enc[:, BLOC * k:BLOC * (k + 1), :])
        in_maps.append(m)
    return in_maps


def _get_program():
    if "nc" not in _CACHE:
        _CACHE["nc"] = _build_program()
    return _CACHE["nc"]


def _install_ntff_hook():
    """Make run_bass_kernel_spmd(trace=True) work under axon in this image:
    register the ctypes NTFF profile hook that the boot path skips when
    antenv.axon_hooks is absent, and stub out the artifact bucket upload."""
    import types

    if _CACHE.get("hook_done"):
        return
    _CACHE["hook_done"] = True
    try:
        import antenv
        from concourse import bass_utils

        bass_utils.upload_artifacts = lambda tmpdir: tmpdir  # no bucket in container
        if "antenv.axon_hooks" not in sys.modules:
            mod = types.ModuleType("antenv.axon_hooks")
            state = {}
            mod.set_axon_ntff_profile_hook = lambda h: state.__setitem__("h", h)
            mod.get_axon_ntff_profile_hook = lambda: state.get("h")
            sys.modules["antenv.axon_hooks"] = mod
            antenv.axon_hooks = mod
        if "/root/.axon_site" not in sys.path:
            sys.path.insert(0, "/root/.axon_site")
        from antenv.axon_hooks import set_axon_ntff_profile_hook
        from trn_agent_boot.trn_boot import _ntff_profile_via_ctypes

        set_axon_ntff_profile_hook(
            _ntff_profile_via_ctypes("/opt/axon/libaxon_pjrt.so"))
    except Exception as exc:  # pragma: no cover
        print(f"ntff hook install failed ({exc}); tracing unavailable")


def kernel(**inputs) -> np.ndarray:
    from concourse import bass_utils

    trace = bool(_CACHE.get("trace"))
    if trace:
        _install_ntff_hook()
    nc = _get_program()
    in_maps = _prep_inputs(inputs)
    res = bass_utils.run_bass_kernel_spmd(nc, in_maps, core_ids=list(range(NCORES)),
                                          trace=trace)
    _CACHE["last_results"] = res
    outs = np.stack([np.asarray(r["out"]) for r in res.results])    # [8, 512, V]
    full = outs.reshape(NCORES, NW, WSTEPS, BLOC, V).transpose(1, 2, 0, 3, 4)
    full = full.reshape(NW * WSTEPS, B, V)[:NSTEP]
    return np.ascontiguousarray(full.astype(np.float32))
